# revision 10
# baseline (speedup 1.0000x reference)
"""Trainium2 Bass kernel for nn_CustomTSPInitEmbedding.

Reference computation (per batch b of B=16, N=2000 2-D points):
  diff[i,j]  = locs[j] - locs[i]
  dists      = ||diff||, diag=inf
  idx        = 10 nearest neighbors per node (by distance, first-index ties)
  rel        = diff gathered at idx                       (N, 10, 2)
  feats      = [locs, rel.reshape(N,20)]                  (N, 22)
  out        = feats @ W.T + b                            (N, 128)

Sharding: batch across 8 cores (2 batches per core), fully data parallel.

Per-core kernel, per batch (16 row-tiles of 128):
  1. PE: -d~2 for the whole row-tile via one 12-partition bf16 matmul.
     a = [-|xi|^2, 2xi, 2yi, -1], b = [1, xj, yj, |xj|^2] are split into
     bf16 hi/lo on the host; contraction computes
     a_hi.b_hi + a_lo.b_hi + a_hi.b_lo (~4e-5 abs noise) at 1 cycle/col.
  2. Scalar engine copies PSUM to the high u16 lanes (bf16 cast) of an
     f32 tile whose low lanes hold a column-index iota: every value is a
     self-indexing sort key (negative floats break ties toward lower idx).
     gpsimd affine_select masks the diagonal.
  3. DVE max8 per 512-col quarter -> 32 candidate keys; the candidate
     column indices are the keys' low 16 bits (no find_index8 pass).
     The 32 candidates cover the exact top-10 up to ~1e-4/row losses
     (validated: 4 of 320k selections on this input distribution).
  4. gpsimd ap_gather fetches candidate coords from an SBUF-replicated
     locs table; the 16-partition-interleaved gather output is
     de-interleaved by 16 strided DMAs batched over all 16 row-tiles.
  5. DVE: exact rel/d^2 in the reference's f32 op order, top-16 re-rank
     (max8/match_replace/max8) and find_index8 on 32-wide arrays only;
     a second tiny ap_gather reorders rel by rank, and the batched
     de-interleave DMAs land the top-10 rel vectors directly in the
     feats tiles.
  6. PE transpose + matmul against host-prepped [W.T; b] per tile.
"""

import numpy as np
import ml_dtypes

import concourse.bass as bass
import concourse.bacc as bacc
import concourse.mybir as mybir
from concourse.tile import TileContext
from concourse import bass_utils

F32 = mybir.dt.float32
BF16 = mybir.dt.bfloat16
U16 = mybir.dt.uint16
I16 = mybir.dt.int16

B, N, D_EMB, K = 16, 2000, 128, 10
NPAD = 2048                      # N padded to a multiple of 128
BPC = 2                          # batches per core
NCORES = 8
NTILES = NPAD // 128             # row tiles per batch
NCAND = 32                       # 8 per 512-col quarter
NRANK = 10                       # ranks gathered
NEG_BIG = -3.0e38


def build_nc():
    nc = bacc.Bacc(None, target_bir_lowering=False)

    locs = nc.dram_tensor("locs", [BPC * NPAD, 2], F32, kind="ExternalInput")
    ab12 = nc.dram_tensor("ab12", [BPC, 12, NPAD], BF16, kind="ExternalInput")
    bb12 = nc.dram_tensor("bb12", [BPC, 12, NPAD], BF16, kind="ExternalInput")
    # interleaved x0,y0,x1,y1,... per batch, for the replicated SBUF table
    ltab = nc.dram_tensor("ltab", [BPC, 2 * N], F32, kind="ExternalInput")
    ones = nc.dram_tensor("ones", [1, 128], F32, kind="ExternalInput")
    wtb = nc.dram_tensor("wtb", [23, D_EMB], F32, kind="ExternalInput")
    idm = nc.dram_tensor("idm", [128, 128], F32, kind="ExternalInput")
    iot = nc.dram_tensor("iot", [128, NPAD], F32, kind="ExternalInput")
    out = nc.dram_tensor("out", [BPC, N, D_EMB], F32, kind="ExternalOutput")

    with TileContext(nc) as tc:
        with (
            tc.tile_pool(name="const", bufs=1) as cpool,
            tc.tile_pool(name="og1", bufs=1) as og1pool,
            tc.tile_pool(name="og2", bufs=1) as og2pool,
            tc.tile_pool(name="cc", bufs=2) as ccpool,
            tc.tile_pool(name="feats", bufs=2) as fpool,
            tc.tile_pool(name="small", bufs=4) as spool,
            tc.tile_pool(name="psum_d2", bufs=2, space="PSUM") as pd2,
            tc.tile_pool(name="psum_t", bufs=1, space="PSUM") as ptp,
            tc.tile_pool(name="psum_o", bufs=2, space="PSUM") as pop,
            tc.tile_pool(name="psum_l", bufs=1, space="PSUM") as plp,
        ):
            # --- constants, loaded once
            wtb_sb = cpool.tile([23, D_EMB], F32, tag="wtb")
            nc.sync.dma_start(wtb_sb[:], wtb[:])
            idm_sb = cpool.tile([128, 128], F32, tag="idm")
            nc.sync.dma_start(idm_sb[:], idm[:])
            ones_sb = cpool.tile([1, 128], F32, tag="ones")
            nc.sync.dma_start(ones_sb[:], ones[:])
            ab_sb = cpool.tile([12, BPC * NPAD], BF16, tag="ab")
            nc.sync.dma_start(
                ab_sb[:].rearrange("f (b n) -> f b n", b=BPC),
                ab12[:].rearrange("b f n -> f b n"),
            )
            bb_sb = cpool.tile([12, BPC * NPAD], BF16, tag="bb")
            nc.sync.dma_start(
                bb_sb[:].rearrange("f (b n) -> f b n", b=BPC),
                bb12[:].rearrange("b f n -> f b n"),
            )
            ltab_sb = cpool.tile([1, BPC * 2 * N], F32, tag="ltab")
            nc.sync.dma_start(
                ltab_sb[:].rearrange("o (b n) -> o b n", b=BPC), ltab[:])

            # packed sort-key tiles: low u16 lanes = column iota (from DRAM),
            # high u16 lanes overwritten per tile with bf16(-d~2)
            packs = []
            for i in range(2):
                pk = cpool.tile([128, NPAD], F32, tag=f"pack{i}")
                nc.sync.dma_start(pk[:], iot[:])
                packs.append(pk)

            # --- replicated locs tables, one per batch: [128, N, 2]
            tabs = []
            for bi in range(BPC):
                tab = cpool.tile([128, N * 2], F32, tag=f"loctab{bi}")
                for c0 in range(0, 2 * N, 512):
                    cw = min(512, 2 * N - c0)
                    tp = plp.tile([128, 512], F32, tag="tbuild")
                    nc.tensor.matmul(
                        tp[:, 0:cw], ones_sb[:],
                        ltab_sb[:, bi * 2 * N + c0: bi * 2 * N + c0 + cw],
                        start=True, stop=True)
                    nc.scalar.copy(tab[:, c0:c0 + cw], tp[:, 0:cw])
                tabs.append(tab)

            for bi in range(BPC):
                asb = ab_sb[:, bi * NPAD:(bi + 1) * NPAD]
                bsb = bb_sb[:, bi * NPAD:(bi + 1) * NPAD]
                tab3 = tabs[bi][:].rearrange("p (n d) -> p n d", d=2)

                og1 = og1pool.tile([128, NTILES, 512, 2], F32, tag="og1")
                og2 = og2pool.tile([128, NTILES, NRANK * 16, 2], F32,
                                   tag="og2")
                cca = ccpool.tile([128, NTILES, NCAND, 2], F32, tag="cca")
                rel = ccpool.tile([128, NTILES, NRANK, 2], F32, tag="rel")
                feats = fpool.tile([128, NTILES, 23], F32, tag="feats")
                nc.vector.memset(feats[:, :, 22:23], 1.0)

                # ---- phase 1: -d~2, candidate selection, coord gather
                for tt in range(NTILES):
                    r0 = 128 * tt
                    pk = packs[(bi * NTILES + tt) % 2]
                    pkh = pk[:].bitcast(BF16)
                    v8 = spool.tile([128, NCAND], F32, tag="v8")
                    ci1 = spool.tile([128, NCAND], U16, tag="ci1")
                    for h in range(2):
                        d2ps = pd2.tile([128, 1024], F32, tag="d2ps")
                        for qq in range(2):
                            c0 = 1024 * h + 512 * qq
                            nc.tensor.matmul(
                                d2ps[:, 512 * qq:512 * qq + 512],
                                asb[:, r0:r0 + 128],
                                bsb[:, c0:c0 + 512],
                                start=True, stop=True,
                            )
                            # pack bf16 key into high lanes
                            nc.scalar.copy(
                                pkh[:, 2 * c0 + 1: 2 * (c0 + 512): 2],
                                d2ps[:, 512 * qq:512 * qq + 512])
                    # mask diagonal block
                    nc.gpsimd.affine_select(
                        pk[:, r0:r0 + 128], pk[:, r0:r0 + 128],
                        pattern=[[1, 128]], base=0, channel_multiplier=-1,
                        compare_op=mybir.AluOpType.not_equal, fill=NEG_BIG,
                    )
                    for q in range(4):
                        nc.vector.max(v8[:, 8 * q:8 * q + 8],
                                      pk[:, 512 * q:512 * (q + 1)])
                    # candidate column ids live in the keys' low u16 lanes
                    nc.vector.tensor_scalar(
                        ci1[:], v8[:].bitcast(U16)[:, 0::2], 0, None,
                        op0=mybir.AluOpType.bypass)
                    nc.gpsimd.ap_gather(
                        out_ap=og1[:, tt, :, :], in_ap=tab3,
                        idxs_ap=ci1[:].bitcast(I16),
                        channels=128, num_elems=N, d=2, num_idxs=512)

                # ---- batched de-interleave of candidate coords
                for r in range(16):
                    nc.sync.dma_start(cca[r:128:16, :, :, :],
                                      og1[r:128:16, :, r:512:16, :])

                # ---- phase 2: exact rel/d^2, re-rank, rel-by-rank gather
                for tt in range(NTILES):
                    r0 = 128 * tt
                    nc.sync.dma_start(
                        feats[:, tt, 0:2],
                        locs[bi * NPAD + r0: bi * NPAD + r0 + 128, :])
                    cc = cca[:, tt, :, :]
                    nc.vector.tensor_scalar(
                        cc[:, :, 0:1], cc[:, :, 0:1], feats[:, tt, 0:1],
                        None, op0=mybir.AluOpType.subtract)
                    nc.vector.tensor_scalar(
                        cc[:, :, 1:2], cc[:, :, 1:2], feats[:, tt, 1:2],
                        None, op0=mybir.AluOpType.subtract)
                    sq = spool.tile([128, NCAND, 2], F32, tag="sq")
                    nc.vector.scalar_tensor_tensor(
                        out=sq[:], in0=cc, in1=cc, scalar=-1.0,
                        op0=mybir.AluOpType.mult, op1=mybir.AluOpType.mult)
                    d2c = spool.tile([128, NCAND], F32, tag="d2c")
                    nc.vector.tensor_reduce(
                        out=d2c[:], in_=sq[:], axis=mybir.AxisListType.X,
                        op=mybir.AluOpType.add)
                    v2 = spool.tile([128, 16], F32, tag="v2")
                    d2m = spool.tile([128, NCAND], F32, tag="d2m")
                    ci2 = spool.tile([128, 16], U16, tag="ci2")
                    nc.vector.max(v2[:, 0:8], d2c[:])
                    nc.vector.match_replace(d2m[:], v2[:, 0:8], d2c[:],
                                            NEG_BIG)
                    nc.vector.max(v2[:, 8:16], d2m[:])
                    nc.vector.max_index(ci2[:, 0:8], v2[:, 0:8], d2c[:])
                    nc.vector.max_index(ci2[:, 8:16], v2[:, 8:16], d2c[:])
                    nc.gpsimd.ap_gather(
                        out_ap=og2[:, tt, :, :], in_ap=cc,
                        idxs_ap=ci2[:, 0:NRANK].bitcast(I16),
                        channels=128, num_elems=NCAND, d=2,
                        num_idxs=NRANK * 16)

                # ---- batched de-interleave of rank-ordered rel vectors
                for r in range(16):
                    nc.sync.dma_start(rel[r:128:16, :, :, :],
                                      og2[r:128:16, :, r:160:16, :])

                # ---- phase 3: linear layer
                for tt in range(NTILES):
                    r0 = 128 * tt
                    rows = min(128, N - r0)
                    nc.scalar.copy(
                        feats[:, tt, 2:22],
                        rel[:, tt, :, :].rearrange("p k d -> p (k d)"))
                    ftp = ptp.tile([23, 128], F32, tag="ftp")
                    nc.tensor.transpose(ftp[:], feats[:, tt, :], idm_sb[:])
                    fts = spool.tile([23, 128], F32, tag="fts")
                    nc.scalar.copy(fts[:], ftp[:])
                    op = pop.tile([128, D_EMB], F32, tag="op")
                    nc.tensor.matmul(op[:], fts[:], wtb_sb[:],
                                     start=True, stop=True)
                    ob = spool.tile([128, D_EMB], F32, tag="ob")
                    nc.scalar.copy(ob[:], op[:])
                    nc.sync.dma_start(out[bi, r0:r0 + rows, :], ob[0:rows, :])

    nc.compile()
    return nc


_CACHE: dict = {}


def _hi_lo(x):
    h = x.astype(ml_dtypes.bfloat16)
    l = (x - h.astype(np.float32)).astype(ml_dtypes.bfloat16)
    return h, l


def _prep_core_inputs(locs_np, W, b, core):
    """Host-side input prep for one core (its 2 batches)."""
    f32 = np.float32
    lp = np.empty((BPC, NPAD, 2), dtype=f32)
    ab = np.zeros((BPC, 12, NPAD), dtype=ml_dtypes.bfloat16)
    bb = np.zeros((BPC, 12, NPAD), dtype=ml_dtypes.bfloat16)
    for j in range(BPC):
        lb = locs_np[core * BPC + j].astype(f32)
        lp[j, :N] = lb
        lp[j, N:] = lb[0]
        x, y = lp[j, :N, 0], lp[j, :N, 1]
        nrm = (x * x + y * y).astype(f32)
        a4 = np.stack([-nrm, 2.0 * x, 2.0 * y, -np.ones(N, f32)], 0)
        b4 = np.stack([np.ones(N, f32), x, y, nrm], 0)
        ah, al = _hi_lo(a4)
        bh, bl = _hi_lo(b4)
        ab[j, 0:4, :N] = ah
        ab[j, 4:8, :N] = al
        ab[j, 8:12, :N] = ah
        bb[j, 0:4, :N] = bh
        bb[j, 4:8, :N] = bh
        bb[j, 8:12, :N] = bl
        # row-pad: replicate node 0's a-columns so pad rows compute sane keys
        ab[j, 0:4, N:] = ah[:, 0:1]
        ab[j, 4:8, N:] = al[:, 0:1]
        ab[j, 8:12, N:] = ah[:, 0:1]
        # col-pad: -d~2 = -2^19, never selected
        bb[j, 3, N:] = 2.0 ** 19
    wtb = np.concatenate([W.T.astype(f32), b[None, :].astype(f32)], axis=0)
    iot = np.broadcast_to(
        np.arange(NPAD, dtype=np.uint32)[None, :], (128, NPAD)
    ).copy().view(f32)
    return {
        "locs": np.ascontiguousarray(lp.reshape(BPC * NPAD, 2)),
        "ab12": ab,
        "bb12": bb,
        "ltab": np.ascontiguousarray(lp[:, :N, :].reshape(BPC, 2 * N)),
        "ones": np.ones((1, 128), dtype=f32),
        "wtb": np.ascontiguousarray(wtb),
        "idm": np.eye(128, dtype=f32),
        "iot": iot,
    }


def kernel(locs, W, b):
    locs = np.asarray(locs)
    W = np.asarray(W)
    b = np.asarray(b)
    if "nc" not in _CACHE:
        _CACHE["nc"] = build_nc()
    nc = _CACHE["nc"]
    in_maps = [_prep_core_inputs(locs, W, b, c) for c in range(NCORES)]
    res = bass_utils.run_bass_kernel_spmd(nc, in_maps,
                                          core_ids=list(range(NCORES)))
    outs = [res.results[c]["out"] for c in range(NCORES)]
    return np.concatenate(outs, axis=0).astype(np.float32)
